# revision 28
# baseline (speedup 1.0000x reference)
"""GCNConv (PyG-faithful, normalize=True, add_self_loops=True) on 8 Trainium2
NeuronCores via Bass/Tile.

Strategy (1D graph/data parallel), v3:
  - Nodes are partitioned across the 8 cores (12500 rows each, padded to
    12544 = 98 blocks of 128).
  - Phase A: each core computes h_k = (dinv*x)_k @ W in bf16 (dinv folded
    host-side), storing the shard in four row-quarters.  After each quarter
    a quarter-wise AllGather replicates it into that quarter's chunk of the
    global message table (quarter-major layout), pipelining the collective
    with compute and with phase-B gathers.
  - Phase B: each core owns 1/8 of the destination nodes.  Edges (incl.
    self-loops) are host-sorted by (dst window of 7 blocks, table chunk,
    dst block); per (window, chunk) section one dma_gather (SWDGE, queue=c)
    fetches g[src] rows (bf16).  Section tails are padded with index 0
    (harmless row) whose sel columns are zero.  One-hot sel tiles (fp8) are
    generated ON-CHIP per window by a single VectorE is_equal over a
    broadcast iota vs per-tile dl columns (dls input, bf16), eliminating
    the 35MB/core sel stream.  TensorE matmuls segment-sum each block's
    messages into PSUM; tiles straddling block boundaries feed both blocks'
    matmuls with separate sel columns.  Epilogue scales by dinv_dst on
    ScalarE, adds bias on VectorE, stores per window.
"""

import sys

if "/opt/trn_rl_repo" not in sys.path:
    sys.path.insert(0, "/opt/trn_rl_repo")

import numpy as np

P = 128          # partitions / tile edge count / feature dim
NCORES = 8
WBLK = 7         # dst blocks per window
NCHUNK = 4       # table chunks == phase-A quarters
MBUFS = 7        # per-(window,chunk) msg section buffers
SBUFS = 8        # per-pass sel buffers
OBUFS = 2


def _pack(x, edge_index, weight, b):
    """Host-side preprocessing: sharding, normalization, quarter-major table
    layout, per-window section packing, masked-dl columns for on-chip sel."""
    import ml_dtypes

    bf16 = ml_dtypes.bfloat16

    x = np.ascontiguousarray(np.asarray(x, dtype=np.float32))
    ei = np.asarray(edge_index)
    weight = np.ascontiguousarray(np.asarray(weight, dtype=np.float32))
    bias = np.asarray(b, dtype=np.float32).reshape(-1)

    n, nin = x.shape
    assert nin == P and weight.shape == (P, P)
    assert n % NCORES == 0
    nb = n // NCORES                      # nodes per core (12500)
    blocks = (nb + P - 1) // P            # blocks per core (98)
    nbp = blocks * P                      # padded nodes per core (12544)
    nw = blocks // WBLK                   # windows (14)
    assert blocks % WBLK == 0

    # quarters of the local shard (in blocks): 25,25,24,24
    qb = [blocks // NCHUNK + (1 if i < blocks % NCHUNK else 0) for i in range(NCHUNK)]
    qrows = [q * P for q in qb]
    qstart = np.concatenate([[0], np.cumsum(qrows)])
    chunk_rows = [NCORES * r for r in qrows]
    assert max(chunk_rows) < 32768

    src = ei[0].astype(np.int64)
    dst = ei[1].astype(np.int64)

    deg = np.bincount(dst, minlength=n).astype(np.float32) + 1.0
    dinv = 1.0 / np.sqrt(deg)

    loop = np.arange(n, dtype=np.int64)
    src_a = np.concatenate([src, loop])
    dst_a = np.concatenate([dst, loop])

    core = dst_a // nb
    dlc = dst_a - core * nb
    blk = dlc >> 7
    dl = (dlc & 127).astype(np.int16)
    win = blk // WBLK
    sub = (blk % WBLK).astype(np.int8)

    score = src_a // nb
    sloc = src_a % nb
    q = np.searchsorted(qstart[1:-1], sloc, side="right")
    rel = (score * np.asarray(qrows)[q] + (sloc - qstart[q])).astype(np.int16)

    # sort by (core, window, chunk, block, src-rel): ascending source order
    # within each group makes the gather's HBM access monotonic per section
    key = ((((core * nw + win) * NCHUNK + q) * WBLK + sub) * 32768 + rel).astype(
        np.int64
    )
    order = np.argsort(key, kind="stable")
    karr, warr, qarr, subarr = core[order], win[order], q[order], sub[order]
    dlarr, relarr = dl[order], rel[order]

    nsec = nw * NCHUNK
    gk = (karr * nsec + warr * NCHUNK + qarr).astype(np.int64)
    gk2 = gk * WBLK + subarr
    cnt_sec = np.bincount(gk, minlength=NCORES * nsec).reshape(NCORES, nw, NCHUNK)
    cnt_sub = np.bincount(gk2, minlength=NCORES * nsec * WBLK).reshape(
        NCORES, nw, NCHUNK, WBLK
    )
    start_sub = np.cumsum(cnt_sub, axis=-1) - cnt_sub        # exclusive cumsum
    end_sub = start_sub + cnt_sub

    cap = cnt_sec.max(axis=0)                                # [nw, NCHUNK]
    t_s = -(-cap // P)

    tile_base = np.zeros((nw, NCHUNK), np.int64)
    wbase = np.zeros(nw + 1, np.int64)
    col = 0
    for w in range(nw):
        wbase[w] = col
        for c in range(NCHUNK):
            tile_base[w, c] = col
            col += int(t_s[w, c])
    s_tiles = int(col)
    wbase[nw] = col
    jmax = int((wbase[1:] - wbase[:-1]).max())

    # per-core packed idx / dl / sub arrays over the static layout
    gs = np.zeros(NCORES * nsec, np.int64)
    gs[1:] = np.cumsum(np.bincount(gk, minlength=NCORES * nsec))[:-1]
    rank = np.arange(gk.size, dtype=np.int64) - gs[gk]
    base_flat = (tile_base * P).reshape(-1)
    pos = base_flat[gk % nsec] + rank

    idx_lin = np.zeros((NCORES, s_tiles * P), np.int16)      # pads gather row 0
    dl_lin = np.full((NCORES, s_tiles * P), -1, np.int16)
    sub_lin = np.full((NCORES, s_tiles * P), -1, np.int8)
    idx_lin[karr, pos] = relarr
    dl_lin[karr, pos] = dlarr
    sub_lin[karr, pos] = subarr

    # static matmul tile ranges per (window, chunk, sub-block)
    anyb = (cnt_sub > 0).any(axis=0)                          # [nw, NCHUNK, WBLK]
    big = np.iinfo(np.int64).max
    lo_b = np.where(
        anyb, np.where(cnt_sub > 0, start_sub, big).min(axis=0) // P, 0
    )
    hi_b = np.where(
        anyb, -(-np.where(cnt_sub > 0, end_sub, 0).max(axis=0) // P), 0
    )

    # sel entries: (w, c, t, b) in program order; per-(window, chunk) ranges
    sel_list = []       # (w, c, t, b)
    wsel_base = np.zeros(nw + 1, np.int64)
    wc_sel_base = np.zeros((nw, NCHUNK), np.int64)
    mm_meta = []        # mm_meta[w][c][b] = list of (t, scol_in_pass)
    for w in range(nw):
        wsel_base[w] = len(sel_list)
        mm_w = [[[] for _ in range(WBLK)] for _ in range(NCHUNK)]
        for c in range(NCHUNK):
            wc_sel_base[w, c] = len(sel_list)
            for bb in range(WBLK):
                for t in range(int(lo_b[w, c, bb]), int(hi_b[w, c, bb])):
                    scol = len(sel_list) - int(wc_sel_base[w, c])
                    mm_w[c][bb].append((t, scol))
                    sel_list.append((w, c, t, bb))
        mm_meta.append(mm_w)
    wsel_base[nw] = len(sel_list)
    s_sel = len(sel_list)
    selw_max = int((wsel_base[1:] - wsel_base[:-1]).max())
    wc_sel_cnt = np.zeros((nw, NCHUNK), np.int64)
    flat = np.append(wc_sel_base.reshape(-1), s_sel)
    wc_sel_cnt = (flat[1:] - flat[:-1]).reshape(nw, NCHUNK)
    svmax = int(wc_sel_cnt.max())
    sel_w = np.asarray([e[0] for e in sel_list], np.int64)
    sel_c = np.asarray([e[1] for e in sel_list], np.int64)
    sel_t = np.asarray([e[2] for e in sel_list], np.int64)
    sel_b = np.asarray([e[3] for e in sel_list], np.int16)

    # verify every real edge is covered by its block's static tile range
    t_of_pos = (pos - base_flat[gk % nsec]) // P
    lo_e = lo_b[warr, qarr, subarr]
    hi_e = hi_b[warr, qarr, subarr]
    assert (t_of_pos >= lo_e).all() and (t_of_pos < hi_e).all()

    # masked dl per sel entry: dls[e, sid] = dl if edge belongs to b else -1
    sel_gt = tile_base[sel_w, sel_c] + sel_t
    epos = sel_gt[:, None] * P + np.arange(P)[None, :]        # [s_sel, P]
    dls = np.empty((NCORES, P, s_sel), np.int8)
    for k in range(NCORES):
        dle = dl_lin[k][epos]                                 # [s_sel, P]
        sbe = sub_lin[k][epos]
        m = (sbe == sel_b[:, None]) & (dle >= 0)
        dls[k] = np.where(m, dle, -1).astype(np.int8).T

    # wrap-16 + replicate to 128 partitions for dma_gather idx layout
    l16 = s_tiles * P // 16
    idx_wr = idx_lin.reshape(NCORES, l16, 16).transpose(0, 2, 1)
    idx_pack = np.ascontiguousarray(np.tile(idx_wr, (1, NCORES, 1)))

    iota = np.ascontiguousarray(
        np.tile(np.arange(P, dtype=np.int8)[None, :], (P, 1))
    )

    # per-core xT (dinv folded, bf16), dinv columns, bias
    xt = np.zeros((NCORES, P, nbp), bf16)
    dinv_t = np.zeros((NCORES, P, blocks), np.float32)
    for k in range(NCORES):
        xs = x[k * nb : (k + 1) * nb] * dinv[k * nb : (k + 1) * nb, None]
        xt[k, :, :nb] = xs.T.astype(bf16)
        dv = np.zeros(nbp, np.float32)
        dv[:nb] = dinv[k * nb : (k + 1) * nb]
        dinv_t[k] = dv.reshape(blocks, P).T
    w_bf = np.ascontiguousarray(weight.astype(bf16))
    bias_rep = np.ascontiguousarray(np.tile(bias[None, :], (P, 1)))

    meta = dict(
        n=n, nb=nb, blocks=blocks, nbp=nbp, nw=nw,
        qb=qb, qrows=qrows, chunk_rows=chunk_rows,
        t_s=t_s, tile_base=tile_base, wbase=wbase, s_tiles=s_tiles,
        jmax=jmax, l16=l16, s_sel=s_sel, selw_max=selw_max,
        wsel_base=wsel_base, mm_meta=mm_meta, tmax_sec=int(t_s.max()),
        wc_sel_base=wc_sel_base, wc_sel_cnt=wc_sel_cnt, svmax=svmax,
    )
    in_maps = [
        {
            "xt": xt[k],
            "w_in": w_bf,
            "bias": bias_rep,
            "dinv": dinv_t[k],
            "idxp": idx_pack[k],
            "dls": dls[k],
            "iota": iota,
        }
        for k in range(NCORES)
    ]
    return meta, in_maps


def _build_program(meta):
    from concourse import bass, bacc, mybir
    import concourse.tile as tile

    blocks = meta["blocks"]
    nbp = meta["nbp"]
    nw = meta["nw"]
    qb = meta["qb"]
    qrows = meta["qrows"]
    chunk_rows = meta["chunk_rows"]
    t_s = meta["t_s"]
    tile_base = meta["tile_base"]
    wbase = meta["wbase"]
    jmax = meta["jmax"]
    l16 = meta["l16"]
    s_sel = meta["s_sel"]
    selw_max = meta["selw_max"]
    wsel_base = meta["wsel_base"]
    mm_meta = meta["mm_meta"]

    f32 = mybir.dt.float32
    bf16 = mybir.dt.bfloat16
    fp8 = mybir.dt.float8e4
    i16 = mybir.dt.int16
    i8 = mybir.dt.int8

    wl16 = [int(wbase[w + 1] - wbase[w]) * 8 for w in range(nw)]
    wl16_max = max(wl16)
    tmax_sec = meta["tmax_sec"]

    nc = bacc.Bacc(num_swdge_queues=4)
    xt_in = nc.declare_dram_parameter("xt", [P, nbp], bf16, isOutput=False)
    w_in = nc.declare_dram_parameter("w_in", [P, P], bf16, isOutput=False)
    bias_in = nc.declare_dram_parameter("bias", [P, P], f32, isOutput=False)
    dinv_in = nc.declare_dram_parameter("dinv", [P, blocks], f32, isOutput=False)
    idx_in = nc.declare_dram_parameter("idxp", [P, l16], i16, isOutput=False)
    dls_in = nc.declare_dram_parameter("dls", [P, s_sel], i8, isOutput=False)
    iota_in = nc.declare_dram_parameter("iota", [P, P], i8, isOutput=False)
    out_ext = nc.declare_dram_parameter("out", [nbp, P], f32, isOutput=True)

    h_q = [nc.dram_tensor(f"h_q{c}", [qrows[c], P], bf16) for c in range(NCHUNK)]
    g_t = [
        nc.dram_tensor(f"g_t{c}", [chunk_rows[c], P], bf16, addr_space="Shared")
        for c in range(NCHUNK)
    ]
    warm_in = nc.dram_tensor("warm_in", [1, P], bf16)
    warm_out = nc.dram_tensor("warm_out", [NCORES, P], bf16, addr_space="Shared")

    with tile.TileContext(nc) as tc:
        # tiny warmup collective to absorb the ncfw first-collective setup
        nc.gpsimd.collective_compute(
            "AllGather",
            mybir.AluOpType.bypass,
            replica_groups=[list(range(NCORES))],
            ins=[warm_in[:]],
            outs=[warm_out[:]],
        )
        with tc.tile_pool(name="const", bufs=1) as cpool:
            w_sb = cpool.tile([P, P], bf16, tag="w")
            nc.sync.dma_start(out=w_sb[:], in_=w_in[:])
            bias_sb = cpool.tile([P, P], f32, tag="bias")
            nc.sync.dma_start(out=bias_sb[:], in_=bias_in[:])
            dinv_sb = cpool.tile([P, blocks], f32, tag="dinv")
            nc.sync.dma_start(out=dinv_sb[:], in_=dinv_in[:])
            iota_sb = cpool.tile([P, P], i8, tag="iota")
            nc.sync.dma_start(out=iota_sb[:], in_=iota_in[:])
            dls_sb = cpool.tile([P, s_sel], i8, tag="dls")
            nc.scalar.dma_start(out=dls_sb[:], in_=dls_in[:])

            # ---- phase A: h = (dinv*x) @ W per quarter + quarter AllGather
            with (
                tc.tile_pool(name="aph", bufs=2) as apool,
                tc.tile_pool(name="psA", bufs=2, space="PSUM") as psA,
            ):
                qs = 0
                for c in range(NCHUNK):
                    rows, qbt = qrows[c], qb[c]
                    xa = apool.tile([P, rows], bf16, tag="xa")
                    nc.sync.dma_start(out=xa[:], in_=xt_in[:, qs : qs + rows])
                    hq = apool.tile([P, qbt, P], bf16, tag="hq")
                    t = 0
                    while t < qbt:
                        g = min(4, qbt - t)
                        ph = psA.tile([P, 4, P], f32, tag="ph")
                        for j in range(g):
                            nc.tensor.matmul(
                                out=ph[:, j, :],
                                lhsT=xa[:, (t + j) * P : (t + j + 1) * P],
                                rhs=w_sb[:],
                                start=True,
                                stop=True,
                            )
                        nc.scalar.activation(
                            out=hq[:, t : t + g, :],
                            in_=ph[:, :g, :],
                            func=mybir.ActivationFunctionType.Copy,
                        )
                        t += g
                    nc.sync.dma_start(
                        out=h_q[c][:].rearrange("(t p) f -> p t f", p=P),
                        in_=hq[:],
                    )
                    nc.gpsimd.collective_compute(
                        "AllGather",
                        mybir.AluOpType.bypass,
                        replica_groups=[list(range(NCORES))],
                        ins=[h_q[c][:]],
                        outs=[g_t[c][:]],
                    )
                    qs += rows

            # ---- phase B: chunk-major sweep.  All windows' chunk-c gathers
            # issue before chunk c+1, so the gpsimd stream only ever waits on
            # collective c (full overlap of gathers with later collectives).
            # Per-pass PSUM accumulates into per-window SBUF accumulators.
            wc_sel_base = meta["wc_sel_base"]
            wc_sel_cnt = meta["wc_sel_cnt"]
            svmax = meta["svmax"]
            with (
                tc.tile_pool(name="msgp", bufs=MBUFS) as mpool,
                tc.tile_pool(name="selp", bufs=SBUFS) as spool,
                tc.tile_pool(name="accp", bufs=nw) as accpool,
                tc.tile_pool(name="outp", bufs=OBUFS) as opool,
                tc.tile_pool(name="psB", bufs=4, space="PSUM") as psB,
            ):
                idx_sb = cpool.tile([P, l16], i16, tag="idx")
                for i in range(4):
                    s = l16 // 4
                    e = l16 if i == 3 else (i + 1) * s
                    nc.scalar.dma_start(
                        out=idx_sb[:, i * s : e], in_=idx_in[:, i * s : e]
                    )
                accs_sb = {}
                for c in range(NCHUNK):
                    for w in range(nw):
                        tc_ = int(t_s[w, c])
                        if tc_ == 0:
                            continue
                        sec0 = int(tile_base[w, c])
                        mt = mpool.tile(
                            [P, tmax_sec, P], bf16, tag="msg", name=f"msg_{w}_{c}"
                        )
                        nc.gpsimd.dma_gather(
                            out_ap=mt[:, :tc_, :],
                            in_ap=g_t[c][:],
                            idxs_ap=idx_sb[:, sec0 * 8 : (sec0 + tc_) * 8],
                            num_idxs=tc_ * P,
                            num_idxs_reg=tc_ * P,
                            elem_size=P,
                            single_packet=False,
                            queue_num=(w + c) % 4,
                        )
                        nsel = int(wc_sel_cnt[w, c])
                        ws0 = int(wc_sel_base[w, c])
                        sv = spool.tile(
                            [P, svmax, P], fp8, tag="selv", name=f"sv_{w}_{c}"
                        )
                        nc.vector.tensor_tensor(
                            out=sv[:, :nsel, :],
                            in0=iota_sb[:].unsqueeze(1).to_broadcast([P, nsel, P]),
                            in1=dls_sb[:, ws0 : ws0 + nsel]
                            .unsqueeze(2)
                            .to_broadcast([P, nsel, P]),
                            op=mybir.AluOpType.is_equal,
                        )
                        ph = psB.tile(
                            [P, WBLK, P], f32, tag="ph", name=f"ph_{w}_{c}"
                        )
                        for bb in range(WBLK):
                            mml = mm_meta[w][c][bb]
                            if not mml:
                                nc.vector.memset(ph[:, bb, :], 0)
                                continue
                            for i, (t, scol) in enumerate(mml):
                                nc.tensor.matmul(
                                    out=ph[:, bb, :],
                                    lhsT=sv[:, scol, :],
                                    rhs=mt[:, t, :],
                                    start=(i == 0),
                                    stop=(i == len(mml) - 1),
                                )
                        if w not in accs_sb:
                            acc = accpool.tile(
                                [P, WBLK, P], f32, tag="accw", name=f"accw_{w}"
                            )
                            accs_sb[w] = acc
                            nc.vector.tensor_copy(out=acc[:], in_=ph[:])
                        else:
                            acc = accs_sb[w]
                            nc.vector.tensor_tensor(
                                out=acc[:], in0=acc[:], in1=ph[:],
                                op=mybir.AluOpType.add,
                            )
                        if c == NCHUNK - 1:
                            osb = opool.tile([P, WBLK, P], f32, tag="osb")
                            for bb in range(WBLK):
                                gb = w * WBLK + bb
                                nc.scalar.activation(
                                    out=osb[:, bb, :],
                                    in_=acc[:, bb, :],
                                    func=mybir.ActivationFunctionType.Copy,
                                    scale=dinv_sb[:, gb : gb + 1],
                                )
                            nc.vector.tensor_tensor(
                                out=osb[:],
                                in0=osb[:],
                                in1=bias_sb[:]
                                .unsqueeze(1)
                                .to_broadcast([P, WBLK, P]),
                                op=mybir.AluOpType.add,
                            )
                            nc.sync.dma_start(
                                out=out_ext[
                                    w * WBLK * P : (w + 1) * WBLK * P, :
                                ].rearrange("(j p) f -> p j f", p=P),
                                in_=osb[:],
                            )

    nc.finalize()
    return nc


def _run(inputs, trace=False, trace_cores=None):
    from concourse.bass_utils import run_bass_kernel_spmd

    meta, in_maps = _pack(**inputs)
    nc = _build_program(meta)
    res = run_bass_kernel_spmd(
        nc,
        in_maps,
        list(range(NCORES)),
        trace=trace,
        trace_cores=trace_cores,
    )
    n, nb = meta["n"], meta["nb"]
    out = np.empty((n, P), np.float32)
    for k in range(NCORES):
        out[k * nb : (k + 1) * nb] = np.asarray(res.results[k]["out"])[:nb]
    return out, res


def kernel(x, edge_index, weight, b):
    out, _ = _run(dict(x=x, edge_index=edge_index, weight=weight, b=b))
    return out


if __name__ == "__main__":
    rng = np.random.default_rng(0)
    n, e = 100000, 1600000
    x = rng.standard_normal((n, P), dtype=np.float32)
    ei = rng.integers(0, n, (2, e)).astype(np.int64)
    w = (rng.standard_normal((P, P)) / np.sqrt(P)).astype(np.float32)
    bb = (rng.standard_normal(P) * 0.02).astype(np.float32)
    out = kernel(x, ei, w, bb)
    print("out", out.shape, out.dtype)
